# revision 8
# baseline (speedup 1.0000x reference)
"""GuidedSparseAttention Trainium2 kernel.

Problem: B=2, C=256, H=W=64 (N=4096 queries), 8 heads x d=32.
Each group of 16 consecutive queries attends to the same 32 gathered keys
(top-2 coarse blocks x 16 pixels).

Sharding: 8 cores = 2 batches x 4 query-chunks (1024 queries = 64 groups each).
The host pre-gathers the raw k/v columns per group (so the device program is
fully SPMD with no data-dependent indices), casts inputs to bf16, and folds the
v/o biases into a single output bias (exact: softmax rows sum to 1).

Per-core device pipeline (all matmuls bf16 -> fp32 PSUM):
  1. q_projT (co, 1024) and k_selT (co, 2048) via W^T on PE; bias via ACT.
  2. V_sel chunks (128 = 4 groups x 32 keys, 256 co) via PE.
  3. Scores per (g,h): 16-way tile_position-packed 32x32 matmuls.
  4. exp on ACT (scale = 1/sqrt(32) folded), denominators via DVE reduce,
     one reciprocal, normalize via DVE broadcast multiply.
  5. A^T via DVE 32x32 block-transpose (blocks stay in place).
  6. AV: 16-way packed matmuls -> x^T (co, n) layout directly.
  7. o_proj + fused bias -> (co, n) output chunk.
"""

import os
import numpy as np
import ml_dtypes

from concourse import bass, bacc, tile, mybir
from concourse.bass_utils import run_bass_kernel_spmd

BF16 = mybir.dt.bfloat16
F32 = mybir.dt.float32

B, C, H, W = 2, 256, 64, 64
N = H * W                 # 4096
NUM_HEADS = 8
HD = C // NUM_HEADS       # 32
RATIO = 4
K_SAMPLES = 2
KLEN = K_SAMPLES * RATIO * RATIO   # 32
GQ = 16                   # queries per group
NCORES = 8
NCHUNK = N // 4           # 1024 queries per core
NG = NCHUNK // GQ         # 64 groups per core
SCALE = float(HD) ** -0.5

TRACE = False             # set by test.py for profiling runs
_RESULT_CACHE = {}


def _build_program():
    nc = bacc.Bacc("TRN2", target_bir_lowering=False, debug=False)

    # DRAM I/O (per-core shapes)
    d_q = nc.dram_tensor("q", [C, NCHUNK], BF16, kind="ExternalInput")
    d_ks = nc.dram_tensor("ks", [C, NG * KLEN], BF16, kind="ExternalInput")
    d_vs = nc.dram_tensor("vs", [C, NG * KLEN], BF16, kind="ExternalInput")
    d_wq = nc.dram_tensor("wq", [C, C], BF16, kind="ExternalInput")
    d_wk = nc.dram_tensor("wk", [C, C], BF16, kind="ExternalInput")
    d_wv = nc.dram_tensor("wv", [C, C], BF16, kind="ExternalInput")
    d_wo = nc.dram_tensor("wo", [C, C], BF16, kind="ExternalInput")
    d_bq = nc.dram_tensor("bq", [C, 1], F32, kind="ExternalInput")
    d_bk = nc.dram_tensor("bk", [C, 1], F32, kind="ExternalInput")
    d_bo2 = nc.dram_tensor("bo2", [C, 1], F32, kind="ExternalInput")
    d_out = nc.dram_tensor("out", [C, NCHUNK], F32, kind="ExternalOutput")

    AF = mybir.ActivationFunctionType
    AX = mybir.AxisListType

    with tile.TileContext(nc) as tc:
        with (
            tc.tile_pool(name="sb", bufs=1) as sb,
            tc.tile_pool(name="ps", bufs=3, space="PSUM") as ps,
        ):
            # ---- persistent SBUF tiles ----
            q_raw = [sb.tile([128, NCHUNK], BF16, tag=f"q{i}", name=f"q{i}") for i in range(2)]
            ks_raw = [sb.tile([128, NG * KLEN], BF16, tag=f"ks{i}", name=f"ks{i}") for i in range(2)]
            vs_raw = [sb.tile([128, NG * KLEN], BF16, tag=f"vs{i}", name=f"vs{i}") for i in range(2)]
            wq_sb = [sb.tile([128, C], BF16, tag=f"wq{i}", name=f"wq{i}") for i in range(2)]
            wk_sb = [sb.tile([128, C], BF16, tag=f"wk{i}", name=f"wk{i}") for i in range(2)]
            wv_sb = [sb.tile([128, C], BF16, tag=f"wv{i}", name=f"wv{i}") for i in range(2)]
            wo_sb = [sb.tile([128, C], BF16, tag=f"wo{i}", name=f"wo{i}") for i in range(2)]
            bq_sb = [sb.tile([128, 1], F32, tag=f"bq{i}", name=f"bq{i}") for i in range(2)]
            bk_sb = [sb.tile([128, 1], F32, tag=f"bk{i}", name=f"bk{i}") for i in range(2)]
            bo2_sb = [sb.tile([128, 1], F32, tag=f"bo{i}", name=f"bo{i}") for i in range(2)]
            qT = [sb.tile([128, NCHUNK], BF16, tag=f"qT{i}", name=f"qT{i}") for i in range(2)]
            vsel = [sb.tile([128, C + 8], BF16, tag=f"vs_p{c}", name=f"vs_p{c}") for c in range(16)]
            xT = [sb.tile([128, NCHUNK], BF16, tag=f"xT{i}", name=f"xT{i}") for i in range(2)]
            outst = [sb.tile([128, NCHUNK], F32, tag=f"os{i}", name=f"os{i}") for i in range(2)]

            # ---- load inputs ----
            for i in range(2):
                sl = slice(i * 128, (i + 1) * 128)
                nc.sync.dma_start(out=q_raw[i][:], in_=d_q[sl, :])
                nc.sync.dma_start(out=ks_raw[i][:], in_=d_ks[sl, :])
                nc.sync.dma_start(out=vs_raw[i][:], in_=d_vs[sl, :])
                nc.sync.dma_start(out=wq_sb[i][:], in_=d_wq[sl, :])
                nc.sync.dma_start(out=wk_sb[i][:], in_=d_wk[sl, :])
                nc.sync.dma_start(out=wv_sb[i][:], in_=d_wv[sl, :])
                nc.sync.dma_start(out=wo_sb[i][:], in_=d_wo[sl, :])
                nc.sync.dma_start(out=bq_sb[i][:], in_=d_bq[sl, :])
                nc.sync.dma_start(out=bk_sb[i][:], in_=d_bk[sl, :])
                nc.sync.dma_start(out=bo2_sb[i][:], in_=d_bo2[sl, :])

            # ---- q projection: out (co, n) = Wq^T @ q_seqT ----
            for coh in range(2):
                for nch in range(2):
                    pt = ps.tile([128, 512], F32, tag="ps", name="pst")
                    csl = slice(nch * 512, (nch + 1) * 512)
                    for cih in range(2):
                        nc.tensor.matmul(
                            pt[:], wq_sb[cih][:, coh * 128:(coh + 1) * 128],
                            q_raw[cih][:, csl], start=(cih == 0), stop=(cih == 1))
                    nc.scalar.activation(
                        qT[coh][:, csl], pt[:], AF.Identity, bias=bq_sb[coh][:])

            # ---- k projection scattered into zero-padded kz (head-slot cols) ----
            kz = [sb.tile([128, 8192], BF16, tag=f"kz{i}", name=f"kz{i}") for i in range(2)]
            for hb in range(2):
                nc.vector.memset(kz[hb][:], 0.0)
            for hb in range(2):
                for nch in range(4):
                    pt = ps.tile([128, 512], F32, tag="ps", name="pst")
                    csl = slice(nch * 512, (nch + 1) * 512)
                    for cih in range(2):
                        nc.tensor.matmul(
                            pt[:], wk_sb[cih][:, hb * 128:(hb + 1) * 128],
                            ks_raw[cih][:, csl], start=(cih == 0), stop=(cih == 1))
                    for s in range(4):
                        nc.scalar.activation(
                            kz[hb][32 * s:32 * s + 32,
                                   s * 2048 + nch * 512:s * 2048 + (nch + 1) * 512],
                            pt[32 * s:32 * s + 32, :], AF.Identity,
                            bias=bk_sb[hb][32 * s:32 * s + 32, :])

            # ---- V_sel projection (+ ones column for denominators) ----
            for c in range(16):
                pt = ps.tile([128, C], F32, tag="ps", name="pst")
                for cih in range(2):
                    nc.tensor.matmul(
                        pt[:], vs_raw[cih][:, c * 128:(c + 1) * 128],
                        wv_sb[cih][:], start=(cih == 0), stop=(cih == 1))
                nc.vector.tensor_copy(vsel[c][:, 0:C], pt[:])
                nc.vector.memset(vsel[c][:, C:C + 8], 1.0)

            # ---- scores: one MM per (g, hb), free = 4 heads x 32 keys ----
            apall = [sb.tile([32, 8192], BF16, tag=f"apall{i}", name=f"apall{i}")
                     for i in range(2)]
            atall = [sb.tile([32, 8192], BF16, tag=f"atall{i}", name=f"atall{i}")
                     for i in range(2)]
            for hb in range(2):
                nc.vector.memset(apall[hb][:], 0.0)
            for hb in range(2):
                for gq in range(16):
                    pt = ps.tile([16, 512], F32, tag="ps", name="pst")
                    for j in range(4):
                        g = gq * 4 + j
                        nc.tensor.matmul(
                            pt[0:16, j * 128:(j + 1) * 128],
                            qT[hb][:, g * 16:(g + 1) * 16],
                            kz[hb][:].rearrange("p (s c) -> p s c", s=4)[:, :, g * 32:(g + 1) * 32],
                            start=(j == 0), stop=(j == 3))
                    nc.scalar.activation(
                        apall[hb][0:16, gq * 512:(gq + 1) * 512], pt[:], AF.Exp,
                        scale=SCALE)
                nc.vector.transpose(atall[hb][:], apall[hb][:])

            # ---- build zero-padded A^T (AzAll) via class-batched SBUF DMAs ----
            azall = [[sb.tile([128, 1024], BF16, tag=f"az{hb}{hi}", name=f"az{hb}{hi}")
                      for hi in range(4)] for hb in range(2)]
            for hb in range(2):
                for hi in range(4):
                    nc.vector.memset(azall[hb][hi][:], 0.0)
                    for s in range(4):
                        src_ap = atall[hb][0:32, :].rearrange(
                            "p (t rest) -> p t rest", t=16)[:, :, 128 * s + 32 * hi:
                                                            128 * s + 32 * hi + 16]
                        dst_ap = azall[hb][hi][32 * s:32 * s + 32, :].rearrange(
                            "p (t rest) -> p t rest", t=16)[:, :, 16 * s:16 * s + 16]
                        nc.sync.dma_start(out=dst_ap, in_=src_ap)

            # ---- AV + denominators: per group-quad t ----
            for t in range(16):
                pt = ps.tile([64, 264], F32, tag="ps", name="pst")
                first, last = (0, 0), (1, 3)
                for hb in range(2):
                    for hi in range(4):
                        h = hb * 4 + hi
                        nc.tensor.matmul(
                            pt[0:64, h * 32:(h + 1) * 32],
                            azall[hb][hi][:, t * 64:(t + 1) * 64],
                            vsel[t][:, h * 32:(h + 1) * 32],
                            start=(hb, hi) == first, stop=False,
                            skip_group_check=True)
                for hb in range(2):
                    for hi in range(4):
                        h = hb * 4 + hi
                        nc.tensor.matmul(
                            pt[0:64, C + h:C + h + 1],
                            azall[hb][hi][:, t * 64:(t + 1) * 64],
                            vsel[t][:, C:C + 1],
                            start=False, stop=(hb, hi) == last,
                            skip_group_check=True)
                rec_t = sb.tile([64, 8], F32, tag="rec_t", name="rec_t")
                xs_t = sb.tile([64, 256], BF16, tag="xs_t", name="xs_t")
                nc.vector.reciprocal(rec_t[:], pt[0:64, C:C + 8])
                nc.vector.tensor_mul(
                    xs_t[:].rearrange("p (h d) -> p h d", h=8),
                    pt[0:64, 0:C].rearrange("p (h d) -> p h d", h=8),
                    rec_t[:].unsqueeze(2).broadcast_to([64, 8, 32]))
                for coh in range(2):
                    nc.sync.dma_start(
                        out=xT[coh][:, t * 64:(t + 1) * 64],
                        in_=xs_t[:, coh * 128:(coh + 1) * 128], transpose=True)

            # ---- o_proj + fused bias ----
            for coh in range(2):
                for qc in range(2):
                    pt = ps.tile([128, 512], F32, tag="ps", name="pst")
                    csl = slice(qc * 512, (qc + 1) * 512)
                    for cih in range(2):
                        nc.tensor.matmul(
                            pt[:],
                            wo_sb[cih][:, coh * 128:(coh + 1) * 128],
                            xT[cih][:, csl],
                            start=(cih == 0),
                            stop=(cih == 1),
                        )
                    nc.scalar.activation(
                        outst[coh][:, csl], pt[:], AF.Identity, bias=bo2_sb[coh][:]
                    )

            for coh in range(2):
                nc.sync.dma_start(
                    out=d_out[coh * 128:(coh + 1) * 128, :], in_=outst[coh][:]
                )

    nc.finalize()
    return nc


def _sparse_indices_np(cam):
    """Replicate reference._sparse_indices (per-group row, before repeat)."""
    Bm, n_low, _ = cam.shape
    w_low = W // RATIO
    # stable descending sort == jax.lax.top_k tie-breaking (lowest index first)
    topk = np.argsort(-cam, axis=-1, kind="stable")[..., :K_SAMPLES]
    r0 = (topk // w_low) * RATIO
    c0 = (topk % w_low) * RATIO
    dr, dc = np.meshgrid(np.arange(RATIO), np.arange(RATIO), indexing="ij")
    dr = dr.reshape(-1)
    dc = dc.reshape(-1)
    rows = r0[..., None] + dr
    cols = c0[..., None] + dc
    return (rows * W + cols).reshape(Bm, n_low, -1)  # (B, 256, 32)


def kernel(q_high_feat, k_high_feat, v_high_feat, coarse_attn_map,
           Wq, bq, Wk, bk, Wv, bv, Wo, bo):
    q_high_feat = np.asarray(q_high_feat, dtype=np.float32)
    k_high_feat = np.asarray(k_high_feat, dtype=np.float32)
    v_high_feat = np.asarray(v_high_feat, dtype=np.float32)
    coarse_attn_map = np.asarray(coarse_attn_map, dtype=np.float32)
    Wq, Wk, Wv, Wo = (np.asarray(w, dtype=np.float32) for w in (Wq, Wk, Wv, Wo))
    bq, bk, bv, bo = (np.asarray(b, dtype=np.float32) for b in (bq, bk, bv, bo))

    bf = ml_dtypes.bfloat16
    qs = q_high_feat.reshape(B, C, N)
    ks = k_high_feat.reshape(B, C, N)
    vs = v_high_feat.reshape(B, C, N)
    idx = _sparse_indices_np(coarse_attn_map)          # (B, 256, 32)
    bo2 = (bo + bv @ Wo).astype(np.float32)

    in_maps = []
    for core in range(NCORES):
        b, ch = divmod(core, 4)
        gsl = idx[b, ch * NG:(ch + 1) * NG].reshape(-1)  # (NG*KLEN,)
        in_maps.append({
            "q": qs[b, :, ch * NCHUNK:(ch + 1) * NCHUNK].astype(bf),
            "ks": ks[b][:, gsl].astype(bf),
            "vs": vs[b][:, gsl].astype(bf),
            "wq": Wq.astype(bf), "wk": Wk.astype(bf),
            "wv": Wv.astype(bf), "wo": Wo.astype(bf),
            "bq": bq.reshape(C, 1), "bk": bk.reshape(C, 1),
            "bo2": bo2.reshape(C, 1),
        })

    nc = _RESULT_CACHE.get("nc")
    if nc is None:
        nc = _build_program()
        _RESULT_CACHE["nc"] = nc

    res = run_bass_kernel_spmd(nc, in_maps, list(range(NCORES)), trace=TRACE)
    _RESULT_CACHE["last"] = res

    out = np.zeros((B, C, N), dtype=np.float32)
    for core in range(NCORES):
        b, ch = divmod(core, 4)
        out[b, :, ch * NCHUNK:(ch + 1) * NCHUNK] = np.asarray(
            res.results[core]["out"], dtype=np.float32
        )
    return out.reshape(B, C, H, W)


# revision 9
# speedup vs baseline: 1.0256x; 1.0256x over previous
"""GuidedSparseAttention Trainium2 kernel.

Problem: B=2, C=256, H=W=64 (N=4096 queries), 8 heads x d=32.
Each group of 16 consecutive queries attends to the same 32 gathered keys
(top-2 coarse blocks x 16 pixels).

Sharding: 8 cores = 2 batches x 4 query-chunks (1024 queries = 64 groups each).
The host pre-gathers the raw k/v columns per group (so the device program is
fully SPMD with no data-dependent indices), casts inputs to bf16, and folds the
v/o biases into a single output bias (exact: softmax rows sum to 1).

Per-core device pipeline (all matmuls bf16 -> fp32 PSUM):
  1. q_projT (co, 1024) and k_selT (co, 2048) via W^T on PE; bias via ACT.
  2. V_sel chunks (128 = 4 groups x 32 keys, 256 co) via PE.
  3. Scores per (g,h): 16-way tile_position-packed 32x32 matmuls.
  4. exp on ACT (scale = 1/sqrt(32) folded), denominators via DVE reduce,
     one reciprocal, normalize via DVE broadcast multiply.
  5. A^T via DVE 32x32 block-transpose (blocks stay in place).
  6. AV: 16-way packed matmuls -> x^T (co, n) layout directly.
  7. o_proj + fused bias -> (co, n) output chunk.
"""

import os
import numpy as np
import ml_dtypes

from concourse import bass, bacc, tile, mybir
from concourse.bass_utils import run_bass_kernel_spmd

BF16 = mybir.dt.bfloat16
F32 = mybir.dt.float32

B, C, H, W = 2, 256, 64, 64
N = H * W                 # 4096
NUM_HEADS = 8
HD = C // NUM_HEADS       # 32
RATIO = 4
K_SAMPLES = 2
KLEN = K_SAMPLES * RATIO * RATIO   # 32
GQ = 16                   # queries per group
NCORES = 8
NCHUNK = N // 4           # 1024 queries per core
NG = NCHUNK // GQ         # 64 groups per core
SCALE = float(HD) ** -0.5

TRACE = False             # set by test.py for profiling runs
_RESULT_CACHE = {}


def _build_program():
    nc = bacc.Bacc("TRN2", target_bir_lowering=False, debug=False)

    # DRAM I/O (per-core shapes)
    d_q = nc.dram_tensor("q", [C, NCHUNK], BF16, kind="ExternalInput")
    d_ks = nc.dram_tensor("ks", [C, NG * KLEN], BF16, kind="ExternalInput")
    d_vs = nc.dram_tensor("vs", [C, NG * KLEN], BF16, kind="ExternalInput")
    d_wq = nc.dram_tensor("wq", [C, C], BF16, kind="ExternalInput")
    d_wk = nc.dram_tensor("wk", [C, C], BF16, kind="ExternalInput")
    d_wv = nc.dram_tensor("wv", [C, C], BF16, kind="ExternalInput")
    d_wo = nc.dram_tensor("wo", [C, C], BF16, kind="ExternalInput")
    d_bq = nc.dram_tensor("bq", [C, 1], F32, kind="ExternalInput")
    d_bk = nc.dram_tensor("bk", [C, 1], F32, kind="ExternalInput")
    d_bo2 = nc.dram_tensor("bo2", [C, 1], F32, kind="ExternalInput")
    d_out = nc.dram_tensor("out", [C, NCHUNK], F32, kind="ExternalOutput")

    AF = mybir.ActivationFunctionType
    AX = mybir.AxisListType

    with tile.TileContext(nc) as tc:
        with (
            tc.tile_pool(name="sb", bufs=1) as sb,
            tc.tile_pool(name="ps", bufs=4, space="PSUM") as ps,
        ):
            # ---- persistent SBUF tiles ----
            q_raw = [sb.tile([128, NCHUNK], BF16, tag=f"q{i}", name=f"q{i}") for i in range(2)]
            ks_raw = [sb.tile([128, NG * KLEN], BF16, tag=f"ks{i}", name=f"ks{i}") for i in range(2)]
            vs_raw = [sb.tile([128, NG * KLEN], BF16, tag=f"vs{i}", name=f"vs{i}") for i in range(2)]
            wq_sb = [sb.tile([128, C], BF16, tag=f"wq{i}", name=f"wq{i}") for i in range(2)]
            wk_sb = [sb.tile([128, C], BF16, tag=f"wk{i}", name=f"wk{i}") for i in range(2)]
            wv_sb = [sb.tile([128, C], BF16, tag=f"wv{i}", name=f"wv{i}") for i in range(2)]
            wo_sb = [sb.tile([128, C], BF16, tag=f"wo{i}", name=f"wo{i}") for i in range(2)]
            bq_sb = [sb.tile([128, 1], F32, tag=f"bq{i}", name=f"bq{i}") for i in range(2)]
            bk_sb = [sb.tile([128, 1], F32, tag=f"bk{i}", name=f"bk{i}") for i in range(2)]
            bo2_sb = [sb.tile([128, 1], F32, tag=f"bo{i}", name=f"bo{i}") for i in range(2)]
            qT = [sb.tile([128, NCHUNK], BF16, tag=f"qT{i}", name=f"qT{i}") for i in range(2)]
            vsel = [sb.tile([128, C + 8], BF16, tag=f"vs_p{c}", name=f"vs_p{c}") for c in range(16)]
            xT = [sb.tile([128, NCHUNK], BF16, tag=f"xT{i}", name=f"xT{i}") for i in range(2)]
            outst = [sb.tile([128, NCHUNK], F32, tag=f"os{i}", name=f"os{i}") for i in range(2)]

            # ---- load inputs ----
            for i in range(2):
                sl = slice(i * 128, (i + 1) * 128)
                nc.sync.dma_start(out=wq_sb[i][:], in_=d_wq[sl, :])
                nc.sync.dma_start(out=wk_sb[i][:], in_=d_wk[sl, :])
                nc.sync.dma_start(out=wv_sb[i][:], in_=d_wv[sl, :])
                nc.sync.dma_start(out=wo_sb[i][:], in_=d_wo[sl, :])
                nc.sync.dma_start(out=bq_sb[i][:], in_=d_bq[sl, :])
                nc.sync.dma_start(out=bk_sb[i][:], in_=d_bk[sl, :])
                nc.sync.dma_start(out=bo2_sb[i][:], in_=d_bo2[sl, :])
            for i in range(2):
                sl = slice(i * 128, (i + 1) * 128)
                nc.sync.dma_start(out=q_raw[i][:], in_=d_q[sl, :])
            for i in range(2):
                sl = slice(i * 128, (i + 1) * 128)
                nc.sync.dma_start(out=ks_raw[i][:], in_=d_ks[sl, :])
            for i in range(2):
                sl = slice(i * 128, (i + 1) * 128)
                nc.sync.dma_start(out=vs_raw[i][:], in_=d_vs[sl, :])

            # ---- q projection: out (co, n) = Wq^T @ q_seqT ----
            for coh in range(2):
                for nch in range(2):
                    pt = ps.tile([128, 512], F32, tag="ps", name="pst")
                    csl = slice(nch * 512, (nch + 1) * 512)
                    for cih in range(2):
                        nc.tensor.matmul(
                            pt[:], wq_sb[cih][:, coh * 128:(coh + 1) * 128],
                            q_raw[cih][:, csl], start=(cih == 0), stop=(cih == 1))
                    nc.scalar.activation(
                        qT[coh][:, csl], pt[:], AF.Identity, bias=bq_sb[coh][:])

            # ---- k projection scattered into zero-padded kz (head-slot cols) ----
            kz = [sb.tile([128, 8192], BF16, tag=f"kz{i}", name=f"kz{i}") for i in range(2)]
            for hb in range(2):
                nc.vector.memset(kz[hb][:], 0.0)
            for hb in range(2):
                for nch in range(4):
                    pt = ps.tile([128, 512], F32, tag="ps", name="pst")
                    csl = slice(nch * 512, (nch + 1) * 512)
                    for cih in range(2):
                        nc.tensor.matmul(
                            pt[:], wk_sb[cih][:, hb * 128:(hb + 1) * 128],
                            ks_raw[cih][:, csl], start=(cih == 0), stop=(cih == 1))
                    for s in range(4):
                        nc.scalar.activation(
                            kz[hb][32 * s:32 * s + 32,
                                   s * 2048 + nch * 512:s * 2048 + (nch + 1) * 512],
                            pt[32 * s:32 * s + 32, :], AF.Identity,
                            bias=bk_sb[hb][32 * s:32 * s + 32, :])

            # ---- V_sel projection (+ ones column for denominators) ----
            for c in range(16):
                pt = ps.tile([128, C], F32, tag="ps", name="pst")
                for cih in range(2):
                    nc.tensor.matmul(
                        pt[:], vs_raw[cih][:, c * 128:(c + 1) * 128],
                        wv_sb[cih][:], start=(cih == 0), stop=(cih == 1))
                nc.vector.tensor_copy(vsel[c][:, 0:C], pt[:])
                nc.vector.memset(vsel[c][:, C:C + 8], 1.0)

            # ---- scores: one MM per (g, hb), free = 4 heads x 32 keys ----
            apall = [sb.tile([32, 8192], BF16, tag=f"apall{i}", name=f"apall{i}")
                     for i in range(2)]
            atall = [sb.tile([32, 8192], BF16, tag=f"atall{i}", name=f"atall{i}")
                     for i in range(2)]
            for hb in range(2):
                nc.vector.memset(apall[hb][:], 0.0)
            for hb in range(2):
                for gq in range(16):
                    pt = ps.tile([16, 512], F32, tag="ps", name="pst")
                    for j in range(4):
                        g = gq * 4 + j
                        nc.tensor.matmul(
                            pt[0:16, j * 128:(j + 1) * 128],
                            qT[hb][:, g * 16:(g + 1) * 16],
                            kz[hb][:].rearrange("p (s c) -> p s c", s=4)[:, :, g * 32:(g + 1) * 32],
                            start=(j == 0), stop=(j == 3))
                    nc.scalar.activation(
                        apall[hb][0:16, gq * 512:(gq + 1) * 512], pt[:], AF.Exp,
                        scale=SCALE)
                    if gq % 4 == 3:
                        qd = gq // 4
                        nc.vector.transpose(
                            atall[hb][0:32, qd * 2048:(qd + 1) * 2048],
                            apall[hb][0:32, qd * 2048:(qd + 1) * 2048])

            # ---- build zero-padded A^T (AzAll) via class-batched SBUF DMAs ----
            azall = [[sb.tile([128, 1024], BF16, tag=f"az{hb}{hi}", name=f"az{hb}{hi}")
                      for hi in range(4)] for hb in range(2)]
            for hb in range(2):
                for hi in range(4):
                    nc.vector.memset(azall[hb][hi][:], 0.0)
                    for s in range(4):
                        src_ap = atall[hb][0:32, :].rearrange(
                            "p (t rest) -> p t rest", t=16)[:, :, 128 * s + 32 * hi:
                                                            128 * s + 32 * hi + 16]
                        dst_ap = azall[hb][hi][32 * s:32 * s + 32, :].rearrange(
                            "p (t rest) -> p t rest", t=16)[:, :, 16 * s:16 * s + 16]
                        nc.sync.dma_start(out=dst_ap, in_=src_ap)

            # ---- AV + denominators: per group-quad t ----
            for t in range(16):
                pt = ps.tile([64, 264], F32, tag="ps", name="pst")
                first, last = (0, 0), (1, 3)
                for hb in range(2):
                    for hi in range(4):
                        h = hb * 4 + hi
                        nc.tensor.matmul(
                            pt[0:64, h * 32:(h + 1) * 32],
                            azall[hb][hi][:, t * 64:(t + 1) * 64],
                            vsel[t][:, h * 32:(h + 1) * 32],
                            start=(hb, hi) == first, stop=False,
                            skip_group_check=True)
                for hb in range(2):
                    for hi in range(4):
                        h = hb * 4 + hi
                        nc.tensor.matmul(
                            pt[0:64, C + h:C + h + 1],
                            azall[hb][hi][:, t * 64:(t + 1) * 64],
                            vsel[t][:, C:C + 1],
                            start=False, stop=(hb, hi) == last,
                            skip_group_check=True)
                rec_t = sb.tile([64, 8], F32, tag="rec_t", name="rec_t")
                xs_t = sb.tile([64, 256], BF16, tag="xs_t", name="xs_t")
                nc.vector.reciprocal(rec_t[:], pt[0:64, C:C + 8])
                nc.vector.tensor_mul(
                    xs_t[:].rearrange("p (h d) -> p h d", h=8),
                    pt[0:64, 0:C].rearrange("p (h d) -> p h d", h=8),
                    rec_t[:].unsqueeze(2).broadcast_to([64, 8, 32]))
                for coh in range(2):
                    nc.sync.dma_start(
                        out=xT[coh][:, t * 64:(t + 1) * 64],
                        in_=xs_t[:, coh * 128:(coh + 1) * 128], transpose=True)

            # ---- o_proj + fused bias ----
            for coh in range(2):
                for qc in range(2):
                    pt = ps.tile([128, 512], F32, tag="ps", name="pst")
                    csl = slice(qc * 512, (qc + 1) * 512)
                    for cih in range(2):
                        nc.tensor.matmul(
                            pt[:],
                            wo_sb[cih][:, coh * 128:(coh + 1) * 128],
                            xT[cih][:, csl],
                            start=(cih == 0),
                            stop=(cih == 1),
                        )
                    nc.scalar.activation(
                        outst[coh][:, csl], pt[:], AF.Identity, bias=bo2_sb[coh][:]
                    )

            for coh in range(2):
                nc.sync.dma_start(
                    out=d_out[coh * 128:(coh + 1) * 128, :], in_=outst[coh][:]
                )

    nc.finalize()
    return nc


def _sparse_indices_np(cam):
    """Replicate reference._sparse_indices (per-group row, before repeat)."""
    Bm, n_low, _ = cam.shape
    w_low = W // RATIO
    # stable descending sort == jax.lax.top_k tie-breaking (lowest index first)
    topk = np.argsort(-cam, axis=-1, kind="stable")[..., :K_SAMPLES]
    r0 = (topk // w_low) * RATIO
    c0 = (topk % w_low) * RATIO
    dr, dc = np.meshgrid(np.arange(RATIO), np.arange(RATIO), indexing="ij")
    dr = dr.reshape(-1)
    dc = dc.reshape(-1)
    rows = r0[..., None] + dr
    cols = c0[..., None] + dc
    return (rows * W + cols).reshape(Bm, n_low, -1)  # (B, 256, 32)


def kernel(q_high_feat, k_high_feat, v_high_feat, coarse_attn_map,
           Wq, bq, Wk, bk, Wv, bv, Wo, bo):
    q_high_feat = np.asarray(q_high_feat, dtype=np.float32)
    k_high_feat = np.asarray(k_high_feat, dtype=np.float32)
    v_high_feat = np.asarray(v_high_feat, dtype=np.float32)
    coarse_attn_map = np.asarray(coarse_attn_map, dtype=np.float32)
    Wq, Wk, Wv, Wo = (np.asarray(w, dtype=np.float32) for w in (Wq, Wk, Wv, Wo))
    bq, bk, bv, bo = (np.asarray(b, dtype=np.float32) for b in (bq, bk, bv, bo))

    bf = ml_dtypes.bfloat16
    qs = q_high_feat.reshape(B, C, N)
    ks = k_high_feat.reshape(B, C, N)
    vs = v_high_feat.reshape(B, C, N)
    idx = _sparse_indices_np(coarse_attn_map)          # (B, 256, 32)
    bo2 = (bo + bv @ Wo).astype(np.float32)

    in_maps = []
    for core in range(NCORES):
        b, ch = divmod(core, 4)
        gsl = idx[b, ch * NG:(ch + 1) * NG].reshape(-1)  # (NG*KLEN,)
        in_maps.append({
            "q": qs[b, :, ch * NCHUNK:(ch + 1) * NCHUNK].astype(bf),
            "ks": ks[b][:, gsl].astype(bf),
            "vs": vs[b][:, gsl].astype(bf),
            "wq": Wq.astype(bf), "wk": Wk.astype(bf),
            "wv": Wv.astype(bf), "wo": Wo.astype(bf),
            "bq": bq.reshape(C, 1), "bk": bk.reshape(C, 1),
            "bo2": bo2.reshape(C, 1),
        })

    nc = _RESULT_CACHE.get("nc")
    if nc is None:
        nc = _build_program()
        _RESULT_CACHE["nc"] = nc

    res = run_bass_kernel_spmd(nc, in_maps, list(range(NCORES)), trace=TRACE)
    _RESULT_CACHE["last"] = res

    out = np.zeros((B, C, N), dtype=np.float32)
    for core in range(NCORES):
        b, ch = divmod(core, 4)
        out[b, :, ch * NCHUNK:(ch + 1) * NCHUNK] = np.asarray(
            res.results[core]["out"], dtype=np.float32
        )
    return out.reshape(B, C, H, W)
